# revision 3
# baseline (speedup 1.0000x reference)
"""MoE layer (B=2,S=2048,D=1024,H=4096,E=8,K=2) on 8 trn2 NeuronCores.

Strategy: expert-parallel. Host computes the (tiny) gate in float64 numpy,
routes each token to its top-2 experts, gathers+transposes each expert's
token batch, and ships expert e's tokens + weights to core e. Each core
runs the FFN (relu(x@w1)@w2) as two fp16 matmul chains with fp32 PSUM
accumulation. Host scatter-adds the per-expert outputs back with the
re-softmaxed top-2 gate weights.
"""

import sys

sys.path.insert(0, "/opt/trn_rl_repo")

from contextlib import ExitStack

import numpy as np

import concourse.bacc as bacc
import concourse.mybir as mybir
from concourse import bass_utils, tile

B, S, D, H, E, TOPK = 2, 2048, 1024, 4096, 8, 2
P = 128
KD = D // P   # 8  k-tiles over D
KH = H // P   # 32 k-tiles over H
ND = D // P   # 8  out-tiles over D
CTILE = 512

_cache: dict = {}


def _ctiles(C):
    out = []
    c0 = 0
    while c0 < C:
        cw = min(CTILE, C - c0)
        out.append((c0, cw))
        c0 += cw
    return out


def _build(C):
    """One-expert FFN: yt[D,C] = (relu(xt.T @ w1) @ w2).T, all fp16 in, fp32 out."""
    f16 = mybir.dt.float16
    f32 = mybir.dt.float32
    nc = bacc.Bacc("TRN2", target_bir_lowering=False, debug=False, num_devices=E)
    xt = nc.dram_tensor("xt", [D, C], f16, kind="ExternalInput")          # x_e^T
    w1p = nc.dram_tensor("w1p", [KH, P, KD * P], f16, kind="ExternalInput")
    w2p = nc.dram_tensor("w2p", [ND, P, KH * P], f16, kind="ExternalInput")
    yt = nc.dram_tensor("yt", [D, C], f32, kind="ExternalOutput")         # y_e^T

    cts = _ctiles(C)
    with tile.TileContext(nc) as tc, ExitStack() as ctx:
        xpool = ctx.enter_context(tc.tile_pool(name="xt", bufs=KD))
        hpool = ctx.enter_context(tc.tile_pool(name="ht", bufs=KH))
        w1pool = ctx.enter_context(tc.tile_pool(name="w1", bufs=3))
        w2pool = ctx.enter_context(tc.tile_pool(name="w2", bufs=3))
        opool = ctx.enter_context(tc.tile_pool(name="out", bufs=4))
        const = ctx.enter_context(tc.tile_pool(name="const", bufs=1))
        ps1 = ctx.enter_context(tc.tile_pool(name="ps1", bufs=4, space="PSUM"))
        ps2 = ctx.enter_context(tc.tile_pool(name="ps2", bufs=4, space="PSUM"))

        bias0 = const.tile([P, 1], f32)
        nc.any.memset(bias0[:], 0.0)

        xts = []
        for kb in range(KD):
            t = xpool.tile([P, C], f16, tag="xt", name=f"xt{kb}")
            nc.sync.dma_start(t[:], xt.ap()[kb * P : (kb + 1) * P, :])
            xts.append(t)

        hts = [hpool.tile([P, C], f16, tag="ht", name=f"ht{i}") for i in range(KH)]

        # mm1: hT[h-tile] = relu(sum_kb w1[kb,h].T @ xT[kb])
        for h in range(KH):
            w1t = w1pool.tile([P, KD * P], f16, tag="w1")
            nc.sync.dma_start(w1t[:], w1p.ap()[h, :, :])
            for c0, cw in cts:
                pt = ps1.tile([P, CTILE], f32, tag="ps1")
                for kb in range(KD):
                    nc.tensor.matmul(
                        pt[:, :cw],
                        w1t[:, kb * P : (kb + 1) * P],
                        xts[kb][:, c0 : c0 + cw],
                        start=(kb == 0),
                        stop=(kb == KD - 1),
                    )
                nc.scalar.activation(
                    hts[h][:, c0 : c0 + cw],
                    pt[:, :cw],
                    mybir.ActivationFunctionType.Relu,
                    bias=bias0[:],
                )

        # mm2: yT[d-tile] = sum_kb w2[kb,d].T @ hT[kb]
        for d in range(ND):
            w2t = w2pool.tile([P, KH * P], f16, tag="w2")
            nc.sync.dma_start(w2t[:], w2p.ap()[d, :, :])
            for c0, cw in cts:
                pt = ps2.tile([P, CTILE], f32, tag="ps2")
                for kb in range(KH):
                    nc.tensor.matmul(
                        pt[:, :cw],
                        w2t[:, kb * P : (kb + 1) * P],
                        hts[kb][:, c0 : c0 + cw],
                        start=(kb == 0),
                        stop=(kb == KH - 1),
                    )
                ot = opool.tile([P, CTILE], f32, tag="out")
                nc.vector.tensor_copy(ot[:, :cw], pt[:, :cw])
                nc.sync.dma_start(yt.ap()[d * P : (d + 1) * P, c0 : c0 + cw], ot[:, :cw])
    nc.compile()
    return nc


def _gate(xf, gate_w, gate_b):
    """float64 gate: immune to fp32 backend noise for any token whose
    top-2 margin exceeds ~1e-9 (actual min margin is ~7e-6)."""
    logits = xf.astype(np.float64) @ gate_w.astype(np.float64) + gate_b.astype(np.float64)
    m = logits.max(axis=1, keepdims=True)
    p = np.exp(logits - m)
    probs = p / p.sum(axis=1, keepdims=True)
    order = np.argsort(-probs, axis=1, kind="stable")
    top_i = order[:, :TOPK]
    rows = np.arange(xf.shape[0])[:, None]
    top_v = probs[rows, top_i]
    tm = top_v.max(axis=1, keepdims=True)
    tw = np.exp(top_v - tm)
    top_w = tw / tw.sum(axis=1, keepdims=True)
    return probs, top_i, top_w


def kernel(x, gate_w, gate_b, w1, b1, w2, b2, _run_kwargs=None):
    x = np.asarray(x, dtype=np.float32)
    gate_w = np.asarray(gate_w, dtype=np.float32)
    gate_b = np.asarray(gate_b, dtype=np.float32)
    w1 = np.asarray(w1, dtype=np.float32)
    b1 = np.asarray(b1, dtype=np.float32)
    w2 = np.asarray(w2, dtype=np.float32)
    b2 = np.asarray(b2, dtype=np.float32)

    b, s, d = x.shape
    xf = x.reshape(-1, d)
    t = xf.shape[0]

    probs, top_i, top_w = _gate(xf, gate_w, gate_b)

    # aux loss, mirroring the reference in fp32
    probs32 = probs.astype(np.float32)
    importance = probs32.mean(0)
    load = (probs32 > 0).astype(np.float32).mean(0)
    aux_loss = np.float32((importance * load).sum() * E)

    # per-expert token lists (token order), capacity = padded max count
    ids = [np.where((top_i == e).any(axis=1))[0] for e in range(E)]
    wts = []
    for e in range(E):
        sel = top_i[ids[e]]
        kpos = np.where(sel[:, 0] == e, 0, 1)
        wts.append(top_w[ids[e], kpos].astype(np.float32))
    counts = np.array([len(i) for i in ids])
    C = max(256, int(np.ceil(counts.max() / P) * P))

    key = C
    if key not in _cache:
        _cache[key] = _build(C)
    nc = _cache[key]

    in_maps = []
    for e in range(E):
        xte = np.zeros((D, C), dtype=np.float16)
        xte[:, : counts[e]] = xf[ids[e]].T.astype(np.float16)
        w1p = (
            w1[e]
            .reshape(KD, P, KH, P)
            .transpose(2, 1, 0, 3)
            .reshape(KH, P, KD * P)
            .astype(np.float16)
        )
        w2p = (
            w2[e]
            .reshape(KH, P, ND, P)
            .transpose(2, 1, 0, 3)
            .reshape(ND, P, KH * P)
            .astype(np.float16)
        )
        in_maps.append({"xt": xte, "w1p": w1p, "w2p": w2p})

    res = bass_utils.run_bass_kernel_spmd(
        nc, in_maps, core_ids=list(range(E)), **(_run_kwargs or {})
    )
    global last_results
    last_results = res

    out = np.zeros((t, d), dtype=np.float32)
    for e in range(E):
        ye = res.results[e]["yt"][:, : counts[e]].T  # [count, D]
        out[ids[e]] += wts[e][:, None] * (ye + b2[e][None, :])

    return out.reshape(b, s, d), aux_loss
